# revision 17
# baseline (speedup 1.0000x reference)
"""AdaptiveScaleDecoupledMamba on 8 TRN2 NeuronCores.

Sharding: data-parallel over N (16 -> 2 per core); params replicated.
Layout inside a core: channels on partitions, time along the free dim.
The selective scan uses the DVE tensor_tensor_scan instruction
(h_t = a_t*h_{t-1} + b_t per partition along the free dim), one scan per
(state s, d-tile, n, direction). The backward direction runs in flipped
time order produced by an anti-causal conv with reversed reads.
"""
import sys, os
for p in ("/opt/trn_rl_repo", "/root/.axon_site/_ro/trn_rl_repo"):
    if os.path.isdir(p) and p not in sys.path:
        sys.path.append(p)

import numpy as np
import ml_dtypes

import concourse.bass as bass
import concourse.tile as tile
from concourse import mybir
from concourse.bass_utils import run_bass_kernel_spmd

F32 = mybir.dt.float32
BF16 = mybir.dt.bfloat16
AF = mybir.ActivationFunctionType
OP = mybir.AluOpType
bf = ml_dtypes.bfloat16

N, L, DM, DI, DS, KC, RK, NSC = 16, 512, 512, 1024, 16, 4, 32, 3
NLOC = N // 8          # 2 sequences per core
ROWS = NLOC * L        # 1024
PADW = L + 6           # per-n padded width in xs tiles (3 left + 3 right)
EPS = 1e-5

_CACHE = {}


def _split_waits(nc):
    """Codegen on this path allows only ONE embedded sync wait per
    instruction; hoist extras into preceding NoOps."""
    ctr = 0
    for fn in nc.m.functions:
        for bb in fn.blocks:
            new = []
            for inst in bb.instructions:
                si = inst.sync_info
                ow = list(si.on_wait) if (si and si.on_wait) else []
                if len(ow) > 1:
                    for w in ow[:-1]:
                        ctr += 1
                        new.append(mybir.InstNoOp(
                            name=f"wsplit_{ctr}", engine=inst.engine,
                            ins=[], outs=[],
                            sync_info=mybir.SyncInfo(on_wait=[w], on_update=[])))
                    si.on_wait = ow[-1:]
                    inst.sync_info = si
                new.append(inst)
            bb.instructions = new


def build():
    nc = bass.Bass()

    def dpar(name, shape, dt=F32):
        return nc.declare_dram_parameter(name, list(shape), dt, isOutput=False)

    # --- per-core shards
    x_d = dpar("x", (NLOC, L, DM))
    hi_d = dpar("h_init", (NLOC, DM))
    # --- replicated params (host-prepped layouts)
    wgfT = dpar("WgfT", (DM, 128), BF16)
    bgf = dpar("bgf", (128,))
    se_d = dpar("se", (128,), BF16)
    w1T = dpar("W1T", (256, DM), BF16)
    b1_d = dpar("b1", (DM,))
    w2T = dpar("W2T", (DM, DM), BF16)
    b2_d = dpar("b2", (DM,))
    w3T = dpar("W3T", (DM, 3 * DM), BF16)
    b3_d = dpar("b3", (3 * DM,))
    lng = dpar("lng", (3, DM))   # per-group LN gamma (constants folded)
    lnb = dpar("lnb", (3, DM))   # per-group LN beta
    winT = dpar("WinT", (DM, 2 * DI), BF16)
    woutT = dpar("WoutT", (DI, DM), BF16)
    ident_d = dpar("ident", (128, 128))
    identb_d = dpar("identb", (128, 128), BF16)
    ones_d = dpar("ones", (128, 1))
    convw = [dpar(f"convw{d}", (DI, KC)) for d in range(2)]
    convb = [dpar(f"convb{d}", (DI,)) for d in range(2)]
    xprojT = [dpar(f"xprojT{d}", (DI, RK + 2 * DS), BF16) for d in range(2)]
    dtwT = [dpar(f"dtwT{d}", (RK, DI), BF16) for d in range(2)]
    dtb = [dpar(f"dtb{d}", (DI,)) for d in range(2)]
    A_d = [dpar(f"A{d}", (DI, DS)) for d in range(2)]
    Dp_d = [dpar(f"Dp{d}", (DI,)) for d in range(2)]
    resw = dpar("resw", (128, 1))

    out_d = nc.declare_dram_parameter("out", [NLOC, L, DM], F32, isOutput=True)
    res_d = nc.declare_dram_parameter("residual", [NLOC, L, DM], F32, isOutput=True)

    # internal scratch
    rows_dram = nc.dram_tensor("rows_scratch", [3, NLOC, DM], F32)  # dA'/dB/dC'' rows
    dbl_dram = [nc.dram_tensor(f"dbl_scratch{d}", [RK + 2 * DS, ROWS], BF16)
                for d in range(2)]
    z_dram = nc.dram_tensor("z_scratch", [8, 128, ROWS], BF16)

    from contextlib import ExitStack
    with tile.TileContext(nc) as tc, ExitStack() as es:
        cp = es.enter_context(tc.tile_pool(name="const", bufs=1))
        big = es.enter_context(tc.tile_pool(name="big", bufs=1))
        xrp = es.enter_context(tc.tile_pool(name="xrp", bufs=1))
        ps_mm = es.enter_context(tc.tile_pool(name="ps_mm", bufs=2, space=bass.MemorySpace.PSUM))
        ps_y = es.enter_context(tc.tile_pool(name="ps_y", bufs=2, space=bass.MemorySpace.PSUM))
        ps_sm = es.enter_context(tc.tile_pool(name="ps_sm", bufs=3, space=bass.MemorySpace.PSUM))
        wk = es.enter_context(tc.tile_pool(name="wk", bufs=2))
        wk3 = es.enter_context(tc.tile_pool(name="wk3", bufs=3))  # scan working set

        dma = nc.gpsimd.dma_start

        def cload(name, src, shape, dt=F32):
            t = cp.tile(list(shape), dt, tag=name)
            dma(t[:], src)
            return t

        # ---------- constants into SBUF
        ident = cload("ident", ident_d[:], (128, 128))
        zero_t = cp.tile([128, 1], F32, tag="zero", name="zero")
        nc.gpsimd.memset(zero_t[:], 0.0)
        eps_t = cp.tile([128, 1], F32, tag="eps", name="eps")
        nc.gpsimd.memset(eps_t[:], EPS)
        one_t = cp.tile([128, 1], F32, tag="one", name="one")
        nc.gpsimd.memset(one_t[:], 1.0)
        identb = cload("identb", identb_d[:], (128, 128), BF16)
        ones_t = cload("ones", ones_d[:], (128, 1))
        _v = wgfT[:].rearrange("(k p) m -> k p m", p=128)
        wgfT_t = [cload(f"pgw{k}", _v[k], (128, 128), BF16) for k in range(4)]
        bgf_t = cload("bgf", bgf[:].rearrange("(a p) -> p a", p=128), (128, 1))
        se_t = cload("se", se_d[:].rearrange("(a p) -> p a", p=128), (128, 1), BF16)
        _v = w1T[:].rearrange("(k p) m -> k p m", p=128)
        w1T_t = [cload(f"pgw{k}", _v[k], (128, DM), BF16) for k in range(2)]
        _v = w2T[:].rearrange("(k p) m -> k p m", p=128)
        w2T_t = [cload(f"pgw{k}", _v[k], (128, DM), BF16) for k in range(4)]
        _v = w3T[:].rearrange("(k p) m -> k p m", p=128)
        w3T_t = [cload(f"pgw{k}", _v[k], (128, 3 * DM), BF16) for k in range(4)]
        b1_t = cload("b1", b1_d[:].rearrange("(c p) -> p c", p=128), (128, 4))
        b2_t = cload("b2", b2_d[:].rearrange("(c p) -> p c", p=128), (128, 4))
        b3_t = cload("b3", b3_d[:].rearrange("(c p) -> p c", p=128), (128, 12))
        lng_t = cload("lng", lng[:].rearrange("g (c p) -> p (g c)", p=128), (128, 12))
        lnb_t = cload("lnb", lnb[:].rearrange("g (c p) -> p (g c)", p=128), (128, 12))
        _v = winT[:].rearrange("(k p) m -> k p m", p=128)
        winT_t = [cload(f"pgw{k}", _v[k], (128, 2 * DI), BF16) for k in range(4)]
        _v = woutT[:].rearrange("(k p) m -> k p m", p=128)
        woutT_t = [cload(f"woutT{k}", _v[k], (128, DM), BF16) for k in range(8)]
        hi_t = [cload(f"hi{n}", hi_d[n:n + 1, :], (1, DM)) for n in range(NLOC)]
        resw_t = cload("resw", resw[:], (128, 1))
        convw_t = [[cload(f"convw{d}_{j}",
                          convw[d][:].rearrange("(j p) k -> j p k", p=128)[j],
                          (128, KC)) for j in range(8)] for d in range(2)]
        convb_t = [[cload(f"convb{d}_{j}",
                          convb[d][:].rearrange("(j p b) -> j p b", p=128, b=1)[j],
                          (128, 1)) for j in range(8)] for d in range(2)]
        xprojT_t = [[cload(f"xprojT{d}_{k}",
                           xprojT[d][:].rearrange("(k p) m -> k p m", p=128)[k],
                           (128, RK + 2 * DS), BF16) for k in range(8)] for d in range(2)]
        dtwT_t = [cload(f"dtwT{d}", dtwT[d][:], (RK, DI), BF16) for d in range(2)]
        dtb_t = [[cload(f"dtb{d}_{j}",
                        dtb[d][:].rearrange("(j p b) -> j p b", p=128, b=1)[j],
                        (128, 1)) for j in range(8)] for d in range(2)]
        A_t = [[cload(f"A{d}_{j}",
                      A_d[d][:].rearrange("(j p) s -> j p s", p=128)[j],
                      (128, DS)) for j in range(8)] for d in range(2)]
        Dp_t = [[cload(f"Dp{d}_{j}",
                       Dp_d[d][:].rearrange("(j p b) -> j p b", p=128, b=1)[j],
                       (128, 1)) for j in range(8)] for d in range(2)]

        xv = x_d[:].rearrange("n (c p) m -> (n c) p m", p=128)  # 8 x [128, DM] row tiles
        resv = res_d[:].rearrange("n (c p) m -> (n c) p m", p=128)
        outv = out_d[:].rearrange("n (c p) m -> (n c) p m", p=128)

        # ---------- stage 1: column means per n  ->  meanT [128, (dc, n)]
        meanT = big.tile([128, 4 * NLOC], BF16, tag="meanT", name="meanT")
        for n in range(NLOC):
            xts = []
            for tcb in range(4):
                xt = xrp.tile([128, DM], F32, tag=f"xr{tcb}", name=f"xr{tcb}")
                dma(xt[:], xv[n * 4 + tcb])
                xts.append(xt)
            for dc in range(4):
                mp = ps_sm.tile([128, 1], F32, tag="sm", name="sm")
                for tcb in range(4):
                    nc.tensor.matmul(mp[:], xts[tcb][:, dc * 128:(dc + 1) * 128],
                                     ones_t[:], start=(tcb == 0), stop=(tcb == 3))
                nc.scalar.activation(meanT[:, dc * NLOC + n: dc * NLOC + n + 1], mp[:],
                                     AF.Copy, scale=1.0 / L)

        # ---------- stage 2: parameter generator (feat-on-partitions, n free)
        def mm_chain(lhsT_tiles, rhs_tiles, m_tiles, tagp):
            outs = []
            nk = len(rhs_tiles)
            for m in range(m_tiles):
                pp = ps_sm.tile([128, NLOC], F32, tag=tagp)
                for k in range(nk):
                    nc.tensor.matmul(pp[:], lhsT_tiles[k][:, m * 128:(m + 1) * 128],
                                     rhs_tiles[k][:], start=(k == 0), stop=(k == nk - 1))
                outs.append(pp)
            return outs

        mean_tiles = [meanT[:, dc * NLOC:(dc + 1) * NLOC] for dc in range(4)]
        gf_ps = mm_chain([wgfT_t[k] for k in range(4)], mean_tiles, 1, "sm")[0]
        gf = wk.tile([128, NLOC], BF16, tag="gf", name="gf")
        nc.scalar.activation(gf[:], gf_ps[:], AF.Gelu, bias=bgf_t[:])
        se2 = wk.tile([128, NLOC], BF16, tag="se2", name="se2")
        for n in range(NLOC):
            nc.vector.tensor_copy(se2[:, n:n + 1], se_t[:])
        cond = [gf[:], se2[:]]
        h1 = []
        h1ps = mm_chain([w1T_t[k] for k in range(2)], cond, 4, "sm")
        for m in range(4):
            t = wk.tile([128, NLOC], BF16, tag=f"h1_{m}", name=f"h1_{m}")
            nc.scalar.activation(t[:], h1ps[m][:], AF.Gelu, bias=b1_t[:, m:m + 1])
            h1.append(t[:])
        h2 = []
        h2ps = mm_chain([w2T_t[k] for k in range(4)], h1, 4, "sm")
        for m in range(4):
            t = wk.tile([128, NLOC], BF16, tag=f"h2_{m}", name=f"h2_{m}")
            nc.scalar.activation(t[:], h2ps[m][:], AF.Gelu, bias=b2_t[:, m:m + 1])
            h2.append(t[:])
        offs = []
        offps = mm_chain([w3T_t[k] for k in range(4)], h2, 12, "sm")
        for m in range(12):
            t = wk.tile([128, NLOC], F32, tag=f"off_{m}", name=f"off_{m}")
            nc.vector.tensor_scalar(t[:], offps[m][:], b3_t[:, m:m + 1], None, OP.add)
            offs.append(t)

        # layer norms per group (A, B, C), feat on partitions
        rows_sb = big.tile([NLOC, 3 * DM], F32, tag="rows_sb", name="rows_sb")
        for g in range(3):
            G = offs[g * 4:(g + 1) * 4]
            sp = ps_sm.tile([1, NLOC], F32, tag="sm", name="sm")
            for c in range(4):
                nc.tensor.matmul(sp[:], ones_t[:], G[c][:], start=(c == 0), stop=(c == 3))
            sqs = []
            for c in range(4):
                t = wk.tile([128, NLOC], F32, tag=f"lnsq{c}", name=f"lnsq{c}")
                nc.vector.tensor_tensor(t[:], G[c][:], G[c][:], OP.mult)
                sqs.append(t)
            sp2 = ps_sm.tile([1, NLOC], F32, tag="sm", name="sm")
            for c in range(4):
                nc.tensor.matmul(sp2[:], ones_t[:], sqs[c][:], start=(c == 0), stop=(c == 3))
            mu = wk.tile([1, NLOC], F32, tag="mu", name="mu")
            nc.scalar.activation(mu[:], sp[:], AF.Copy, scale=1.0 / DM)
            ex2 = wk.tile([1, NLOC], F32, tag="ex2", name="ex2")
            nc.scalar.activation(ex2[:], sp2[:], AF.Copy, scale=1.0 / DM)
            musq = wk.tile([1, NLOC], F32, tag="musq", name="musq")
            nc.vector.tensor_tensor(musq[:], mu[:], mu[:], OP.mult)
            var = wk.tile([1, NLOC], F32, tag="var", name="var")
            nc.vector.tensor_tensor(var[:], ex2[:], musq[:], OP.subtract)
            sd = wk.tile([1, NLOC], F32, tag="sd", name="sd")
            nc.scalar.activation(sd[:], var[:], AF.Sqrt, bias=eps_t[0:1, :])
            inv = wk.tile([1, NLOC], F32, tag="inv", name="inv")
            nc.vector.reciprocal(inv[:], sd[:])
            mi_dram = nc.dram_tensor(f"mi{g}", [2, NLOC], F32)
            nc.sync.dma_start(mi_dram[0:1, :], mu[:])
            nc.sync.dma_start(mi_dram[1:2, :], inv[:])
            mu_b = wk.tile([128, NLOC], F32, tag="mu_b", name="mu_b")
            dma(mu_b[:], mi_dram[0:1, :].partition_broadcast(128))
            inv_b = wk.tile([128, NLOC], F32, tag="inv_b", name="inv_b")
            dma(inv_b[:], mi_dram[1:2, :].partition_broadcast(128))
            for c in range(4):
                cen = wk.tile([128, NLOC], F32, tag="cen", name="cen")
                nc.vector.tensor_tensor(cen[:], G[c][:], mu_b[:], OP.subtract)
                scl = wk.tile([128, NLOC], F32, tag="scl", name="scl")
                nc.vector.tensor_tensor(scl[:], cen[:], inv_b[:], OP.mult)
                fin = wk.tile([128, NLOC], F32, tag="fin", name="fin")
                nc.vector.tensor_scalar(fin[:], scl[:], lng_t[:, g * 4 + c:g * 4 + c + 1],
                                        lnb_t[:, g * 4 + c:g * 4 + c + 1], OP.mult, OP.add)
                tp = ps_sm.tile([NLOC, 128], F32, tag="sm", name="sm")
                nc.tensor.transpose(tp[:], fin[:], ident[:])
                nc.vector.tensor_copy(
                    rows_sb[:, g * DM + c * 128: g * DM + (c + 1) * 128], tp[:])
        nc.sync.dma_start(rows_dram[:].rearrange("g n m -> n g m"),
                          rows_sb[:].rearrange("p (g m) -> p g m", g=3))

        gb = []  # gb[g][n] -> [128, DM] broadcast rows
        for g in range(3):
            per_n = []
            for n in range(NLOC):
                t = big.tile([128, DM], F32, tag=f"gb{g}_{n}", name=f"gb{g}_{n}")
                dma(t[:], rows_dram[g, n:n + 1, :].partition_broadcast(128))
                per_n.append(t)
            gb.append(per_n)

        # ---------- stage 3: residual, rms norm, transpose (streamed per row tile)
        hnT = [big.tile([128, ROWS], BF16, tag=f"y0_{k}", name=f"y0_{k}") for k in range(4)]
        for i in range(8):
            n = i // 4
            rt = wk.tile([128, DM], F32, tag="rt", name="rt")
            dma(rt[:], xv[i])
            nc.vector.tensor_tensor(rt[:], rt[:], gb[1][n][:], OP.add)
            if i % 4 == 0:
                nc.vector.tensor_tensor(rt[0:1, :], rt[0:1, :], hi_t[n][:], OP.add)
            nc.sync.dma_start(resv[i], rt[:])
            sqd = ps_sm.tile([128, DM], F32, tag="sm", name="sqd")
            ssq = wk.tile([128, 1], F32, tag="ssq", name="ssq")
            nc.scalar.activation(sqd[:], rt[:], AF.Square, bias=zero_t[:], accum_out=ssq[:])
            sdr = wk.tile([128, 1], F32, tag="sdr", name="sdr")
            nc.scalar.activation(sdr[:], ssq[:], AF.Sqrt, scale=1.0 / DM, bias=eps_t[:])
            invr = wk.tile([128, 1], F32, tag="invr", name="invr")
            nc.vector.reciprocal(invr[:], sdr[:])
            hn = wk.tile([128, DM], F32, tag="hn", name="hn")
            nc.vector.tensor_scalar(hn[:], rt[:], invr[:], None, OP.mult)
            for k in range(4):
                tp = ps_sm.tile([128, 128], F32, tag="sm", name="sm")
                nc.tensor.transpose(tp[:], hn[:, k * 128:(k + 1) * 128], ident[:])
                nc.vector.tensor_copy(hnT[k][:, i * 128:(i + 1) * 128], tp[:])

        # ---------- stage 4: xz = hn @ W_in.T  (-> xs padded bf16, z silu'd bf16)
        xs_t = [big.tile([128, NLOC * PADW], BF16, tag=f"xs{j}", name=f"xs{j}") for j in range(8)]
        for j in range(8):
            nc.gpsimd.memset(xs_t[j][:], 0.0)
        for j in range(16):
            for nh in range(2):
                pp = ps_mm.tile([128, L], F32, tag="mm", name="mm")
                for k in range(4):
                    nc.tensor.matmul(pp[:], winT_t[k][:, j * 128:(j + 1) * 128],
                                     hnT[k][:, nh * L:(nh + 1) * L],
                                     start=(k == 0), stop=(k == 3))
                if j < 8:
                    nc.vector.tensor_copy(
                        xs_t[j][:, nh * PADW + 3: nh * PADW + 3 + L], pp[:])
                else:
                    zs = wk.tile([128, L], BF16, tag="zs", name="zs")
                    nc.scalar.activation(zs[:], pp[:], AF.Silu, bias=zero_t[:])
                    nc.sync.dma_start(z_dram[j - 8, :, nh * L:(nh + 1) * L], zs[:])

        # ---------- stage 5+6: per direction conv -> dbl -> scan
        yf_t = yb_t = None
        for d in range(2):
            xc_d = {}
            for j in range(8):
                xc = big.tile([128, ROWS], BF16, tag=f"xc{j}", name=f"xc{j}")  # shared across dirs
                for n in range(NLOC):
                    base = n * PADW
                    acc = wk.tile([128, L], BF16, tag="cacc", name="cacc")
                    if d == 0:
                        src0 = xs_t[j][:, base + 0: base + L]
                    else:
                        src0 = xs_t[j][:, base + 6: base + 6 + L][:, ::-1]
                    nc.vector.tensor_scalar(acc[:], src0, convw_t[d][j][:, 0:1],
                                            None, OP.mult)
                    for k in range(1, 4):
                        acc2 = wk.tile([128, L], BF16, tag="cacc", name="cacc")
                        if d == 0:
                            srck = xs_t[j][:, base + k: base + k + L]
                        else:
                            srck = xs_t[j][:, base + 6 - k: base + 6 - k + L][:, ::-1]
                        nc.vector.scalar_tensor_tensor(
                            acc2[:], srck, convw_t[d][j][:, k:k + 1], acc[:],
                            OP.mult, OP.add)
                        acc = acc2
                    nc.scalar.activation(xc[:, n * L:(n + 1) * L], acc[:], AF.Silu,
                                         bias=convb_t[d][j][:])
                xc_d[j] = xc
            # dbl = xproj @ xc   -> [64, ROWS] -> bf16 -> DRAM scratch
            dbl_sb = wk.tile([RK + 2 * DS, ROWS], BF16, tag="dbl_sb", name="dbl_sb")
            for nh in range(2):
                dp = ps_mm.tile([RK + 2 * DS, L], F32, tag="mm", name="mm")
                for k in range(8):
                    nc.tensor.matmul(dp[:], xprojT_t[d][k][:],
                                     xc_d[k][:, nh * L:(nh + 1) * L],
                                     start=(k == 0), stop=(k == 7))
                nc.vector.tensor_copy(dbl_sb[:, nh * L:(nh + 1) * L], dp[:])
            nc.sync.dma_start(dbl_dram[d][:], dbl_sb[:])

            y_t = [big.tile([128, ROWS], BF16, tag=(f"y0_{j}" if d == 0 else f"xs{j}"),
                            name=f"yt{d}_{j}") for j in range(8)]
            for n in range(NLOC):
                Ball = big.tile([128, DS * L], BF16, tag="Ball", name="Ball")
                Call = big.tile([128, DS * L], BF16, tag="Call", name="Call")
                for s in range(DS):
                    dma(Ball[:, s * L:(s + 1) * L],
                        dbl_dram[d][RK + s, n * L:(n + 1) * L].partition_broadcast(128))
                    dma(Call[:, s * L:(s + 1) * L],
                        dbl_dram[d][RK + DS + s, n * L:(n + 1) * L].partition_broadcast(128))
                for j in range(8):
                    dtp = ps_mm.tile([128, L], F32, tag="mm", name="mm")
                    nc.tensor.matmul(dtp[:], dtwT_t[d][:, j * 128:(j + 1) * 128],
                                     dbl_sb[0:RK, n * L:(n + 1) * L],
                                     start=True, stop=True)
                    e1 = wk.tile([128, L], F32, tag="dt", name="e1")
                    nc.scalar.activation(e1[:], dtp[:], AF.Exp, bias=dtb_t[d][j][:])
                    dt = wk.tile([128, L], F32, tag="dt", name="dt")
                    nc.scalar.activation(dt[:], e1[:], AF.Ln, bias=one_t[:])
                    dtu = wk.tile([128, L], BF16, tag="dtu", name="dtu")
                    nc.vector.tensor_tensor(
                        dtu[:], dt[:], xc_d[j][:, n * L:(n + 1) * L], OP.mult)
                    yp = ps_y.tile([128, L], F32, tag="y", name="y")
                    for s in range(DS):
                        a = wk3.tile([128, L], F32, tag="a", name="a")
                        nc.scalar.activation(a[:], dt[:], AF.Exp, bias=zero_t[:],
                                             scale=A_t[d][j][:, s:s + 1])
                        b = wk3.tile([128, L], BF16, tag="b", name="b")
                        nc.vector.tensor_tensor(b[:], dtu[:],
                                                Ball[:, s * L:(s + 1) * L], OP.mult)
                        h = wk3.tile([128, L], BF16, tag="h", name="h")
                        nc.vector.tensor_tensor_scan(h[:], a[:], b[:], 0.0,
                                                     OP.mult, OP.add)
                        cy = wk3.tile([128, L], BF16, tag="cy", name="cy")
                        nc.vector.tensor_tensor(cy[:], h[:],
                                                Call[:, s * L:(s + 1) * L], OP.mult)
                        nc.tensor.matmul(yp[:], identb[:], cy[:],
                                         start=(s == 0), stop=(s == DS - 1))
                    nc.vector.scalar_tensor_tensor(
                        y_t[j][:, n * L:(n + 1) * L],
                        xc_d[j][:, n * L:(n + 1) * L], Dp_t[d][j][:], yp[:],
                        OP.mult, OP.add)
            if d == 0:
                yf_t = y_t
            else:
                yb_t = y_t

        # ---------- stage 7: gating  g = (y_f + flip(y_b)) * silu(z)  (into yf_t)
        for j in range(8):
            for n in range(NLOC):
                g1 = wk.tile([128, L], BF16, tag="cacc", name="cacc")
                nc.vector.tensor_tensor(
                    g1[:], yf_t[j][:, n * L:(n + 1) * L],
                    yb_t[j][:, n * L:(n + 1) * L][:, ::-1], OP.add)
                zr = wk.tile([128, L], BF16, tag="zs", name="zr")
                dma(zr[:], z_dram[j, :, n * L:(n + 1) * L])
                nc.vector.tensor_tensor(yf_t[j][:, n * L:(n + 1) * L], g1[:],
                                        zr[:], OP.mult)

        # ---------- stage 8: out = g @ W_out.T, then modulation + residual
        for rc in range(8):
            n = rc // 4
            op_ = ps_mm.tile([128, DM], F32, tag="mm", name="mm")
            for k in range(8):
                nc.tensor.matmul(op_[:], yf_t[k][:, rc * 128:(rc + 1) * 128],
                                 woutT_t[k][:], start=(k == 0), stop=(k == 7))
            o1 = wk.tile([128, DM], F32, tag="rt", name="rt")
            nc.vector.tensor_tensor(o1[:], op_[:], gb[0][n][:], OP.mult)
            o2 = wk.tile([128, DM], F32, tag="rt2", name="o2")
            nc.vector.tensor_tensor(o2[:], o1[:], gb[2][n][:], OP.add)
            rr = wk.tile([128, DM], F32, tag="hn", name="hn")
            dma(rr[:], resv[rc])
            o3 = wk.tile([128, DM], F32, tag="dt", name="dt")
            nc.vector.scalar_tensor_tensor(o3[:], rr[:], resw_t[:], o2[:],
                                           OP.mult, OP.add)
            nc.sync.dma_start(outv[rc], o3[:])

    _split_waits(nc)
    return nc


def _host_prep(inputs):
    sid = int(np.asarray(inputs["scale_id"]))
    sc = float(np.linspace(0.9, 0.3, NSC)[sid])
    ln_g = np.asarray(inputs["ln_g"], np.float32)
    ln_b = np.asarray(inputs["ln_b"], np.float32)
    # groups: 0 -> dA' = 1 + 0.1*sc*LN(dA):  gamma*0.1*sc, beta*0.1*sc + 1
    #         1 -> dB  = LN(dB)
    #         2 -> dC''= 0.1*LN(dC)
    lng = np.stack([ln_g * (0.1 * sc), ln_g, ln_g * 0.1], axis=0).astype(np.float32)
    lnbm = np.stack([ln_b * (0.1 * sc) + 1.0, ln_b, ln_b * 0.1], axis=0).astype(np.float32)

    shared = {
        "WgfT": np.ascontiguousarray(np.asarray(inputs["W_gf"]).T).astype(bf),
        "bgf": np.asarray(inputs["b_gf"], np.float32),
        "se": np.asarray(inputs["emb"][sid], np.float32),
        "W1T": np.ascontiguousarray(np.asarray(inputs["W1"]).T).astype(bf),
        "b1": np.asarray(inputs["b1"], np.float32),
        "W2T": np.ascontiguousarray(np.asarray(inputs["W2"]).T).astype(bf),
        "b2": np.asarray(inputs["b2"], np.float32),
        "W3T": np.ascontiguousarray(np.asarray(inputs["W3"]).T).astype(bf),
        "b3": np.asarray(inputs["b3"], np.float32),
        "lng": lng, "lnb": lnbm,
        "WinT": np.ascontiguousarray(
            (np.asarray(inputs["W_in"]) * np.asarray(inputs["rms_w"])[None, :]).T
        ).astype(bf),
        "WoutT": np.ascontiguousarray(np.asarray(inputs["W_out"]).T).astype(bf),
        "ident": np.eye(128, dtype=np.float32),
        "identb": np.eye(128).astype(bf),
        "ones": np.ones((128, 1), np.float32),
        "resw": np.full((128, 1), float(np.asarray(inputs["res_w"]).reshape(-1)[0]), np.float32),
    }
    for d, sfx in enumerate(("f", "b")):
        shared[f"convw{d}"] = np.ascontiguousarray(
            np.asarray(inputs[f"convw_{sfx}"])[:, 0, :]).astype(np.float32)
        shared[f"convb{d}"] = np.asarray(inputs[f"convb_{sfx}"], np.float32)
        shared[f"xprojT{d}"] = np.ascontiguousarray(
            np.asarray(inputs[f"xproj_{sfx}"]).T).astype(bf)
        shared[f"dtwT{d}"] = np.ascontiguousarray(
            np.asarray(inputs[f"dtw_{sfx}"]).T).astype(bf)
        shared[f"dtb{d}"] = np.asarray(inputs[f"dtb_{sfx}"], np.float32)
        shared[f"A{d}"] = (-np.exp(np.asarray(inputs[f"Alog_{sfx}"]))).astype(np.float32)
        shared[f"Dp{d}"] = np.asarray(inputs[f"D_{sfx}"], np.float32)
    shared["se"] = shared["se"].astype(bf)
    return shared


def kernel(**inputs):
    if "nc" not in _CACHE:
        _CACHE["nc"] = build()
    nc = _CACHE["nc"]
    shared = _host_prep(inputs)
    x = np.asarray(inputs["x"], np.float32)
    h_init = np.asarray(inputs["h_init"], np.float32)
    in_maps = []
    for c in range(8):
        m = dict(shared)
        m["x"] = np.ascontiguousarray(x[c * NLOC:(c + 1) * NLOC])
        m["h_init"] = np.ascontiguousarray(h_init[c * NLOC:(c + 1) * NLOC])
        in_maps.append(m)
    res = run_bass_kernel_spmd(nc, in_maps, list(range(8))).results
    out = np.concatenate([r["out"] for r in res], axis=0)
    residual = np.concatenate([r["residual"] for r in res], axis=0)
    h_final = out[:, -1, :].copy()
    return out, residual, h_final
